# revision 1
# baseline (speedup 1.0000x reference)
"""LocalGCN message-passing kernel, data-parallel over 8 NeuronCores.

Sharding (per spec hint): pure data parallel — the batch dim (B=32768) is
split into 8 shards of 4096 rows, one per core; all parameters are
replicated; each row's 8-neighbor attention is independent so there is no
cross-core communication. Inputs arrive FULL, are sharded host-side,
executed SPMD on the 8 cores, and the outputs are gathered back to the
full [32768, 128] array.
"""
import os

# Faster neuronxcc compile; must be set before the jax backend initializes.
_flags = os.environ.get("NEURON_CC_FLAGS", "")
if "--optlevel" not in _flags and "-O" not in _flags:
    os.environ["NEURON_CC_FLAGS"] = (_flags + " --optlevel=2").strip()

import numpy as np
import jax
import jax.numpy as jnp
from functools import partial

# Hardcoded problem shape (nn_LocalGCN_70489003262550)
D_IN, HID, HEADS, MAXN, OUT, B = 16, 128, 4, 8, 128, 32768
HD = HID // HEADS
EPS = 1e-5
N_CORES = 8
B_SH = B // N_CORES  # 4096 rows per core

PARAM_NAMES = (
    "enc_w1", "enc_b1", "enc_g1", "enc_be1", "enc_w2", "enc_b2", "enc_g2",
    "enc_be2", "in_proj_w", "in_proj_b", "out_w", "out_b", "an_g", "an_b",
    "p1_w", "p1_b", "p1_g", "p1_be", "p2_w", "p2_b", "p2_g", "p2_be",
)


def _ln(x, g, b):
    m = jnp.mean(x, -1, keepdims=True)
    v = jnp.mean((x - m) ** 2, -1, keepdims=True)
    return (x - m) * jax.lax.rsqrt(v + EPS) * g + b


def _encode(x, p):
    h = _ln(x @ p["enc_w1"].T + p["enc_b1"], p["enc_g1"], p["enc_be1"])
    h = jax.nn.relu(h)
    return _ln(h @ p["enc_w2"].T + p["enc_b2"], p["enc_g2"], p["enc_be2"])


def _shard_forward(current_node, neighbor_nodes, neighbor_mask, *params):
    p = dict(zip(PARAM_NAMES, params))
    bsz = current_node.shape[0]
    cur_emb = _encode(current_node, p)                                # [b, HID]
    nb_emb = _encode(neighbor_nodes, p)                               # [b, N, HID]

    valid = neighbor_mask > 0                                         # [b, N]
    has_nb = jnp.any(valid, axis=1)                                   # [b]

    wq, wk, wv = (p["in_proj_w"][:HID], p["in_proj_w"][HID:2 * HID],
                  p["in_proj_w"][2 * HID:])
    bq, bk, bv = (p["in_proj_b"][:HID], p["in_proj_b"][HID:2 * HID],
                  p["in_proj_b"][2 * HID:])
    q = (cur_emb @ wq.T + bq).reshape(bsz, HEADS, HD)
    k = (nb_emb @ wk.T + bk).reshape(bsz, MAXN, HEADS, HD)
    v = (nb_emb @ wv.T + bv).reshape(bsz, MAXN, HEADS, HD)

    scores = jnp.einsum("bhd,bnhd->bhn", q, k) / np.sqrt(HD)          # [b, H, N]
    safe = jnp.where(has_nb[:, None], valid, True)                    # [b, N]
    scores = jnp.where(safe[:, None, :], scores, -1e9)
    attn = jax.nn.softmax(scores, axis=-1)
    ctx = jnp.einsum("bhn,bnhd->bhd", attn, v).reshape(bsz, HID)
    ctx = ctx @ p["out_w"].T + p["out_b"]
    agg = jnp.where(has_nb[:, None], _ln(ctx, p["an_g"], p["an_b"]), cur_emb)

    combined = jnp.concatenate([cur_emb, agg], axis=-1)               # [b, 2*HID]
    h = _ln(combined @ p["p1_w"].T + p["p1_b"], p["p1_g"], p["p1_be"])
    h = jax.nn.relu(h)
    return _ln(h @ p["p2_w"].T + p["p2_b"], p["p2_g"], p["p2_be"])    # [b, OUT]


_cache = {"fp": None, "fn": None}


def _get_pmapped(params):
    # Bake the (small, replicated) parameters into the executable as
    # constants: avoids 22 params x 8 devices of per-call H2D round trips
    # over the tunneled PJRT link. Re-traces only if param values change.
    fp = hash(tuple(p.tobytes() for p in params))
    if _cache["fp"] != fp:
        const = {n: jnp.asarray(p) for n, p in zip(PARAM_NAMES, params)}

        def fwd(cur, nb, mask):
            return _shard_forward(cur, nb, mask,
                                  *(const[n] for n in PARAM_NAMES))

        _cache["fn"] = jax.pmap(fwd, devices=jax.devices()[:N_CORES])
        _cache["fp"] = fp
    return _cache["fn"]


def kernel(**inputs) -> np.ndarray:
    cur = np.asarray(inputs["current_node"], np.float32).reshape(
        N_CORES, B_SH, D_IN)
    nb = np.asarray(inputs["neighbor_nodes"], np.float32).reshape(
        N_CORES, B_SH, MAXN, D_IN)
    mask = np.asarray(inputs["neighbor_mask"], np.int32).reshape(
        N_CORES, B_SH, MAXN)
    params = tuple(np.asarray(inputs[n], np.float32) for n in PARAM_NAMES)

    fn = _get_pmapped(params)
    out = fn(cur, nb, mask)                      # [8, 4096, OUT]
    return np.asarray(out).reshape(B, OUT)


if __name__ == "__main__":
    rng = np.random.default_rng(0)
    demo = {
        "current_node": rng.standard_normal((B, D_IN), np.float32),
        "neighbor_nodes": rng.standard_normal((B, MAXN, D_IN), np.float32),
        "neighbor_mask": rng.integers(0, 2, (B, MAXN)).astype(np.int32),
    }
    for n in PARAM_NAMES:
        pass  # weights needed; run via test.py instead



# revision 11
# speedup vs baseline: 378.2853x; 378.2853x over previous
"""LocalGCN message-passing kernel for 8 TRN2 NeuronCores (Bass/Tile).

Sharding (per spec hint): pure data parallel — batch dim (B=32768) split into
8 shards of 4096 rows; params replicated; no cross-core communication.

Device math (validated against the reference in numpy, rel err ~1e-3):
- fp16 activations / folded weights, fp32 PSUM accumulation and stats.
- All five LayerNorms use column-CENTERED weights, so each matmul emits
  mean-subtracted pre-activations directly and normalize is a pure per-row
  scale r = rsqrt(E[y_c^2] + eps).
- enc layer1 variance comes from an exact quadratic form in the input
  (extra 17 rhs columns), evaluated by one fused tensor_tensor_reduce.
  Other variances via ScalarE Square with accum_out.
- LN affines ride for free: g1/be1 and p1_g/p1_be inside the ScalarE Relu
  on the feature-major side of the PE transpose; g2/be2 folded into the
  q/k/v projections; an_g/an_b applied feature-major; k-bias dropped
  (softmax shift-invariant); v-bias folded into the out-proj bias.
- Attention: scores via DVE q*k + head-sum matmul; mask folded host-side
  into an additive bias row (accumulated by a K=1 matmul); softmax without
  max-subtraction (scores are O(1)); denominator applied after the
  weighted sum.
"""
import os
import sys

import numpy as np

sys.path.insert(0, "/opt/trn_rl_repo")

D_IN, HID, HEADS, MAXN, OUT, B = 16, 128, 4, 8, 128, 32768
HD = HID // HEADS
EPS = 1e-5
NEG = -30000.0
N_CORES = 8
R = B // N_CORES            # 4096 rows per core
NT = R // 128               # 32 row-tiles per core
NG = MAXN + 1               # 9 encoder groups per row-tile (8 nb + 1 cur)

PARAM_NAMES = (
    "enc_w1", "enc_b1", "enc_g1", "enc_be1", "enc_w2", "enc_b2", "enc_g2",
    "enc_be2", "in_proj_w", "in_proj_b", "out_w", "out_b", "an_g", "an_b",
    "p1_w", "p1_b", "p1_g", "p1_be", "p2_w", "p2_b", "p2_g", "p2_be",
)

WAVES = [[0, 1, 2], [3, 4, 5], [6, 7, 8]]   # g8 = cur, g0..7 = neighbors


# ---------------------------------------------------------------- host folds
def _center_cols(wt, b):
    return wt - wt.mean(axis=1, keepdims=True), b - b.mean()


def _fold(p):
    f = {}
    w1t = p["enc_w1"].T.astype(np.float64)
    w1c, b1c = _center_cols(w1t, p["enc_b1"].astype(np.float64))
    wext = np.vstack([w1c, b1c[None, :]])                    # [17, 128]
    q1 = (wext @ wext.T) / HID                               # [17, 17]
    f["w1rhs"] = np.concatenate([wext, q1], axis=1).astype(np.float16)

    w2c, b2c = _center_cols(p["enc_w2"].T.astype(np.float64),
                            p["enc_b2"].astype(np.float64))
    f["w2rhs"] = w2c.astype(np.float16)
    f["b2row"] = b2c[None, :].astype(np.float16)

    g2 = p["enc_g2"].astype(np.float64)
    be2 = p["enc_be2"].astype(np.float64)
    ipw = p["in_proj_w"].astype(np.float64)
    ipb = p["in_proj_b"].astype(np.float64)
    wq, wk, wv = ipw[:HID], ipw[HID:2 * HID], ipw[2 * HID:]
    bq, bv = ipb[:HID], ipb[2 * HID:]
    sc = 1.0 / np.sqrt(HD)
    f["wq_l"] = ((g2[:, None] * wq.T) * sc).astype(np.float16)
    f["bqrow"] = (((be2 @ wq.T) + bq) * sc)[None, :].astype(np.float16)
    f["wk_l"] = (g2[:, None] * wk.T).astype(np.float16)
    f["wv_l"] = (g2[:, None] * wv.T).astype(np.float16)
    bv_eff = (be2 @ wv.T) + bv

    ow = p["out_w"].astype(np.float64)
    bo = bv_eff @ ow.T + p["out_b"].astype(np.float64)
    owt_c, bo_c = _center_cols(ow.T, bo)
    f["wo_rhs"] = owt_c.astype(np.float16)
    f["borow"] = bo_c[None, :].astype(np.float16)

    p1w = p["p1_w"].astype(np.float64)
    p1at, p1bt = p1w[:, :HID].T, p1w[:, HID:].T
    f["p1a_rhs"] = (p1at - p1at.mean(axis=1, keepdims=True)).astype(np.float16)
    f["p1b_rhs"] = (p1bt - p1bt.mean(axis=1, keepdims=True)).astype(np.float16)
    p1b = p["p1_b"].astype(np.float64)
    f["p1brow"] = (p1b - p1b.mean())[None, :].astype(np.float16)

    p2c, p2bc = _center_cols(p["p2_w"].T.astype(np.float64),
                             p["p2_b"].astype(np.float64))
    f["p2rhs"] = p2c.astype(np.float16)
    f["p2brow"] = p2bc[None, :].astype(np.float16)

    for n in ("enc_g1", "enc_be1", "enc_g2", "enc_be2", "an_g", "an_b",
              "p1_g", "p1_be", "p2_g", "p2_be"):
        f[n] = p[n].astype(np.float32)
    hs = np.zeros((HID, HEADS), np.float16)
    hb = np.zeros((HEADS, HID), np.float16)
    for h in range(HEADS):
        hs[HD * h:HD * (h + 1), h] = 1.0
        hb[h, HD * h:HD * (h + 1)] = 1.0
    f["headsum"] = hs
    f["headbc"] = hb
    f["headbc32"] = hb.astype(np.float32)
    f["ident"] = np.eye(HID, dtype=np.float16)
    return f


def _prep_shard(cur, nb, mask):
    """Per-core data arrays. cur [R,16] f32, nb [R,8,16], mask [R,8] i32."""
    r = cur.shape[0]
    nbf = nb.reshape(r * MAXN, D_IN)
    one_r = np.ones((1, r), np.float32)
    one_n = np.ones((1, r * MAXN), np.float32)
    d = {
        "cur_fm": np.concatenate([cur.T, one_r], 0).astype(np.float16),
        "nb_fm": np.concatenate([nbf.T, one_n], 0).astype(np.float16),
        "cur_rm": np.concatenate([cur, one_r.T], 1).astype(np.float16),
        "nb_rm": np.concatenate([nbf, one_n.T], 1).astype(np.float16),
    }
    valid = mask > 0
    has_nb = valid.any(axis=1)
    safe = np.where(has_nb[:, None], valid, True)
    d["mb"] = np.where(safe, 0.0, NEG).astype(np.float16).reshape(r * MAXN)
    d["sel"] = has_nb.astype(np.float16)
    return d


# ------------------------------------------------------------- bass builder
def _build(nc):
    import concourse.bass as bass
    import concourse.tile as tile
    from concourse import mybir

    f16, f32 = mybir.dt.float16, mybir.dt.float32
    AF = mybir.ActivationFunctionType
    OP = mybir.AluOpType

    # DRAM I/O
    cur_fm = nc.dram_tensor("cur_fm", (D_IN + 1, R), f16, kind="ExternalInput")
    nb_fm = nc.dram_tensor("nb_fm", (D_IN + 1, R * MAXN), f16,
                           kind="ExternalInput")
    cur_rm = nc.dram_tensor("cur_rm", (R, D_IN + 1), f16, kind="ExternalInput")
    nb_rm = nc.dram_tensor("nb_rm", (R * MAXN, D_IN + 1), f16,
                           kind="ExternalInput")
    mb_d = nc.dram_tensor("mb", (R * MAXN,), f16, kind="ExternalInput")
    sel_d = nc.dram_tensor("sel", (R,), f16, kind="ExternalInput")
    wd = {}
    for n, shape in (("w1rhs", (17, 145)), ("w2rhs", (128, 128)),
                     ("b2row", (1, 128)), ("wq_l", (128, 128)),
                     ("bqrow", (1, 128)), ("wk_l", (128, 128)),
                     ("wv_l", (128, 128)), ("wo_rhs", (128, 128)),
                     ("borow", (1, 128)), ("p1a_rhs", (128, 128)),
                     ("p1b_rhs", (128, 128)), ("p1brow", (1, 128)),
                     ("p2rhs", (128, 128)), ("p2brow", (1, 128))):
        wd[n] = nc.dram_tensor(n, shape, f16, kind="ExternalInput")
    vd = {}
    for n in ("enc_g1", "enc_be1", "enc_g2", "enc_be2", "an_g", "an_b",
              "p1_g", "p1_be", "p2_g", "p2_be"):
        vd[n] = nc.dram_tensor(n, (HID,), f32, kind="ExternalInput")
    hs_d = nc.dram_tensor("headsum", (128, HEADS), f16, kind="ExternalInput")
    hb_d = nc.dram_tensor("headbc", (HEADS, 128), f16, kind="ExternalInput")
    hb32_d = nc.dram_tensor("headbc32", (HEADS, 128), f32,
                            kind="ExternalInput")
    id_d = nc.dram_tensor("ident", (128, 128), f16, kind="ExternalInput")
    out_d = nc.dram_tensor("out", (R, OUT), f32, kind="ExternalOutput")

    with tile.TileContext(nc) as tc:
        with (
            tc.tile_pool(name="singles", bufs=1) as singles,
            tc.tile_pool(name="io", bufs=2) as io,
            tc.tile_pool(name="enc", bufs=2) as enc,
            tc.tile_pool(name="att", bufs=2) as att,
            tc.tile_pool(name="small", bufs=3) as small,
            tc.tile_pool(name="ps", bufs=3, space="PSUM") as ps,
        ):
            # ---- persistent constants
            w = {n: singles.tile(list(t.shape), f16, tag=n, name="w_" + n)
                 for n, t in wd.items()}
            for n, t in wd.items():
                nc.sync.dma_start(w[n], t[:])
            v = {n: singles.tile([HID, 1], f32, tag=n, name="v_" + n)
                 for n in vd}
            for n, t in vd.items():
                nc.sync.dma_start(v[n], t[:].rearrange("(a b) -> a b", b=1))
            curfm = singles.tile([D_IN + 1, R], f16, tag="curfm")
            nc.sync.dma_start(curfm, cur_fm[:])

            ident = singles.tile([128, 128], f16, tag="ident")
            nc.sync.dma_start(ident, id_d[:])
            ones1 = singles.tile([1, 128], f16, tag="ones1")
            nc.vector.memset(ones1, 1.0)
            epst = singles.tile([128, 1], f32, tag="eps")
            nc.vector.memset(epst, EPS)
            headsum = singles.tile([128, HEADS], f16, tag="hs")
            nc.sync.dma_start(headsum, hs_d[:])
            headbc = singles.tile([HEADS, 128], f16, tag="hb")
            nc.sync.dma_start(headbc, hb_d[:])
            headbc32 = singles.tile([HEADS, 128], f32, tag="hb32")
            nc.sync.dma_start(headbc32, hb32_d[:])
            p2g_ap = vd["p2_g"][:]
            p2g_bc = singles.tile([128, 128], f32, tag="p2gbc")
            nc.sync.dma_start(p2g_bc, bass.AP(
                tensor=p2g_ap.tensor, offset=p2g_ap.offset,
                ap=[[0, 128]] + list(p2g_ap.ap)))
            p2be_ap = vd["p2_be"][:]
            p2be_bc = singles.tile([128, 128], f32, tag="p2bebc")
            nc.sync.dma_start(p2be_bc, bass.AP(
                tensor=p2be_ap.tensor, offset=p2be_ap.offset,
                ap=[[0, 128]] + list(p2be_ap.ap)))

            for t in range(NT):
                # ---- per-tile input DMAs
                nbfm_t = io.tile([D_IN + 1, 1024], f16, tag="nbfm")
                nc.sync.dma_start(nbfm_t, nb_fm[:, t * 1024:(t + 1) * 1024])
                xn = io.tile([128, MAXN, D_IN + 1], f16, tag="xn")
                nc.sync.dma_start(xn, nb_rm[t * 1024:(t + 1) * 1024, :]
                                  .rearrange("(g b) d -> b g d", g=MAXN))
                xc = io.tile([128, D_IN + 1], f16, tag="xc")
                nc.sync.dma_start(xc, cur_rm[t * 128:(t + 1) * 128, :])
                mb_t = io.tile([1, 1024], f16, tag="mbt")
                nc.sync.dma_start(mb_t, mb_d[t * 1024:(t + 1) * 1024]
                                  .rearrange("(b a) -> b a", b=1))
                sel_t = io.tile([1, 128], f16, tag="selt")
                nc.sync.dma_start(sel_t, sel_d[t * 128:(t + 1) * 128]
                                  .rearrange("(b a) -> b a", b=1))

                def lhs_x(g):
                    if g == MAXN:
                        return curfm[:, t * 128:(t + 1) * 128]
                    return nbfm_t[:, g * 128:(g + 1) * 128]

                # ================= encoder layer 1 =================
                v1t = small.tile([128, NG], f32, tag="v1t")
                sd1 = small.tile([128, NG], f32, tag="sd1")
                r1 = small.tile([128, NG], f32, tag="r1")
                yh1 = enc.tile([128, NG * 128], f16, tag="yh1")
                h1f = enc.tile([128, NG * 128], f16, tag="h1f")
                for wi, wave in enumerate(WAVES):
                    yps = []
                    for g in wave:
                        y1p = ps.tile([128, 145], f32, tag="mm", bufs=3)
                        nc.tensor.matmul(y1p, lhs_x(g), w["w1rhs"],
                                         start=True, stop=True)
                        yps.append(y1p)
                    lo, hi = wave[0], wave[-1] + 1
                    scw = small.tile([128, len(wave), D_IN + 1], f16,
                                     tag="scw")
                    for i, g in enumerate(wave):
                        xin = (xc if g == MAXN else xn[:, g, :])
                        nc.vector.tensor_tensor(
                            out=scw[:, i, :], in0=xin,
                            in1=yps[i][:, 128:145], op=OP.mult)
                    nc.vector.tensor_reduce(v1t[:, lo:hi], scw,
                                            axis=mybir.AxisListType.X,
                                            op=OP.add)
                    nc.scalar.activation(sd1[:, lo:hi], v1t[:, lo:hi],
                                         AF.Sqrt, bias=epst, scale=1.0)
                    nc.vector.reciprocal(r1[:, lo:hi], sd1[:, lo:hi])
                    tp = ps.tile([128, 384], f16, tag="tp", bufs=1)
                    for i, g in enumerate(wave):
                        nc.vector.tensor_scalar_mul(
                            yh1[:, g * 128:(g + 1) * 128],
                            in0=yps[i][:, 0:128], scalar1=r1[:, g:g + 1])
                        nc.tensor.transpose(
                            tp[:, i * 128:(i + 1) * 128],
                            yh1[:, g * 128:(g + 1) * 128], ident)
                    nc.scalar.activation(
                        h1f[:, lo * 128:hi * 128], tp[:, 0:(hi - lo) * 128],
                        AF.Relu, bias=v["enc_be1"], scale=v["enc_g1"])

                # ================= encoder layer 2 =================
                v2t = small.tile([128, NG], f32, tag="v2t")
                sd2 = small.tile([128, NG], f32, tag="sd2")
                r2 = small.tile([128, NG], f32, tag="r2")
                yh2 = enc.tile([128, NG * 128], f16, tag="yh2")
                y2n = enc.tile([128, 1024], f16, tag="y2n")
                yh2c = small.tile([128, 128], f16, tag="yh2c")
                ecf = att.tile([128, 128], f16, tag="ecf")
                for wi, wave in enumerate(WAVES):
                    yps = []
                    for g in wave:
                        y2p = ps.tile([128, 128], f32, tag="mm", bufs=3)
                        nc.tensor.matmul(y2p, h1f[:, g * 128:(g + 1) * 128],
                                         w["w2rhs"], start=True, stop=False)
                        nc.tensor.matmul(y2p, ones1, w["b2row"],
                                         start=False, stop=True)
                        yps.append(y2p)
                        sq = small.tile([128, 128], f16, tag="sq")
                        nc.scalar.activation(sq, y2p, AF.Square,
                                             accum_out=v2t[:, g:g + 1])
                    lo, hi = wave[0], wave[-1] + 1
                    nc.scalar.activation(sd2[:, lo:hi], v2t[:, lo:hi],
                                         AF.Sqrt, bias=epst, scale=1.0 / HID)
                    nc.vector.reciprocal(r2[:, lo:hi], sd2[:, lo:hi])
                    tp = ps.tile([128, 384], f16, tag="tp", bufs=1)
                    for i, g in enumerate(wave):
                        nc.vector.tensor_scalar_mul(
                            yh2[:, g * 128:(g + 1) * 128],
                            in0=yps[i], scalar1=r2[:, g:g + 1])
                        nc.tensor.transpose(
                            tp[:, i * 128:(i + 1) * 128],
                            yh2[:, g * 128:(g + 1) * 128], ident)
                    if wave[-1] < MAXN:       # pure neighbor wave
                        nc.scalar.activation(y2n[:, lo * 128:hi * 128],
                                             tp[:, 0:384], AF.Copy)
                    else:                     # g6, g7 neighbors + g8 cur
                        nc.scalar.activation(y2n[:, lo * 128:MAXN * 128],
                                             tp[:, 0:256], AF.Copy)
                        nc.vector.tensor_copy(yh2c, tp[:, 256:384])
                        nc.vector.tensor_scalar(
                            ecf, in0=tp[:, 256:384], scalar1=v["enc_g2"],
                            scalar2=v["enc_be2"], op0=OP.mult, op1=OP.add)

                # ================= attention =================
                qp = ps.tile([128, 128], f32, tag="mm", bufs=3)
                nc.tensor.matmul(qp, w["wq_l"], yh2c, start=True, stop=False)
                nc.tensor.matmul(qp, w["bqrow"], ones1, start=False, stop=True)
                qs = small.tile([128, 128], f16, tag="qs")
                nc.scalar.activation(qs, qp, AF.Copy)

                kp = ps.tile([128, 1024], f32, tag="big", bufs=2)
                for c in range(2):
                    nc.tensor.matmul(kp[:, c * 512:(c + 1) * 512], w["wk_l"],
                                     y2n[:, c * 512:(c + 1) * 512],
                                     start=True, stop=True)
                qk = att.tile([128, 1024], f16, tag="qk")
                qs_rep = bass.AP(tensor=qs.tensor, offset=qs.offset,
                                 ap=list(qs.ap) + [[0, MAXN]])
                nc.vector.tensor_tensor(
                    out=qk.rearrange("p (b n) -> p b n", n=MAXN),
                    in0=kp.rearrange("p (b n) -> p b n", n=MAXN),
                    in1=qs_rep, op=OP.mult)

                sp = ps.tile([4, 1024], f32, tag="big", bufs=2)
                for c in range(2):
                    nc.tensor.matmul(sp[:, c * 512:(c + 1) * 512], headsum,
                                     qk[:, c * 512:(c + 1) * 512],
                                     start=True, stop=False)
                    nc.tensor.matmul(sp[:, c * 512:(c + 1) * 512],
                                     ones1[:, 0:4],
                                     mb_t[:, c * 512:(c + 1) * 512],
                                     start=False, stop=True)
                ex = att.tile([4, 1024], f16, tag="ex")
                nc.scalar.activation(ex, sp, AF.Exp)
                den = small.tile([4, 128], f32, tag="den")
                nc.vector.tensor_reduce(
                    den, ex.rearrange("p (b n) -> p b n", n=MAXN),
                    axis=mybir.AxisListType.X, op=OP.add)
                rden = small.tile([4, 128], f32, tag="rden")
                nc.vector.reciprocal(rden, den)

                ep = ps.tile([128, 1024], f32, tag="big", bufs=2)
                for c in range(2):
                    nc.tensor.matmul(ep[:, c * 512:(c + 1) * 512], headbc,
                                     ex[:, c * 512:(c + 1) * 512],
                                     start=True, stop=True)
                rdb = ps.tile([128, 128], f32, tag="mm", bufs=3)
                nc.tensor.matmul(rdb, headbc32, rden, start=True, stop=True)

                vp = ps.tile([128, 1024], f32, tag="big", bufs=2)
                for c in range(2):
                    nc.tensor.matmul(vp[:, c * 512:(c + 1) * 512], w["wv_l"],
                                     y2n[:, c * 512:(c + 1) * 512],
                                     start=True, stop=True)
                vs = att.tile([128, 1024], f16, tag="vs")
                nc.scalar.activation(vs, vp, AF.Copy)
                tv = att.tile([128, 1024], f16, tag="tv")
                nc.vector.tensor_tensor(out=tv, in0=ep, in1=vs, op=OP.mult)
                craw = small.tile([128, 128], f32, tag="craw")
                nc.vector.tensor_reduce(
                    craw, tv.rearrange("p (b n) -> p b n", n=MAXN),
                    axis=mybir.AxisListType.X, op=OP.add)
                ctx = small.tile([128, 128], f16, tag="ctx")
                nc.vector.tensor_tensor(out=ctx, in0=rdb, in1=craw,
                                        op=OP.mult)

                # ============ out-proj + an-LN + select ============
                ocp = ps.tile([128, 128], f32, tag="mm", bufs=3)
                nc.tensor.matmul(ocp, ctx, w["wo_rhs"], start=True, stop=False)
                nc.tensor.matmul(ocp, ones1, w["borow"], start=False,
                                 stop=True)
                vo = small.tile([128, 1], f32, tag="vo")
                sqo = small.tile([128, 128], f16, tag="sq")
                nc.scalar.activation(sqo, ocp, AF.Square, accum_out=vo)
                so = small.tile([128, 1], f32, tag="so")
                nc.scalar.activation(so, vo, AF.Sqrt, bias=epst,
                                     scale=1.0 / HID)
                ro = small.tile([128, 1], f32, tag="ro")
                nc.vector.reciprocal(ro, so)
                t_rm = small.tile([128, 128], f16, tag="trm")
                nc.vector.tensor_scalar_mul(t_rm, in0=ocp, scalar1=ro)
                tpo = ps.tile([128, 384], f16, tag="tp", bufs=1)
                nc.tensor.transpose(tpo[:, 0:128], t_rm, ident)
                agg0 = small.tile([128, 128], f16, tag="agg0")
                nc.vector.tensor_scalar(agg0, in0=tpo[:, 0:128],
                                        scalar1=v["an_g"], scalar2=v["an_b"],
                                        op0=OP.mult, op1=OP.add)
                selp = ps.tile([128, 128], f32, tag="mm", bufs=3)
                nc.tensor.matmul(selp, ones1, sel_t, start=True, stop=True)
                selm = small.tile([128, 128], mybir.dt.uint8, tag="selm")
                nc.vector.tensor_copy(selm, selp)
                agg = att.tile([128, 128], f16, tag="agg")
                nc.vector.tensor_copy(agg, ecf)
                nc.vector.copy_predicated(agg, selm, agg0)

                # ================= p1 + LN/relu =================
                z1p = ps.tile([128, 128], f32, tag="mm", bufs=3)
                nc.tensor.matmul(z1p, ecf, w["p1a_rhs"], start=True,
                                 stop=False)
                nc.tensor.matmul(z1p, agg, w["p1b_rhs"], start=False,
                                 stop=False)
                nc.tensor.matmul(z1p, ones1, w["p1brow"], start=False,
                                 stop=True)
                vz = small.tile([128, 1], f32, tag="vz")
                sqz = small.tile([128, 128], f16, tag="sq")
                nc.scalar.activation(sqz, z1p, AF.Square, accum_out=vz)
                sz = small.tile([128, 1], f32, tag="sz")
                nc.scalar.activation(sz, vz, AF.Sqrt, bias=epst,
                                     scale=1.0 / HID)
                rz = small.tile([128, 1], f32, tag="rz")
                nc.vector.reciprocal(rz, sz)
                zh1 = small.tile([128, 128], f16, tag="zh1")
                nc.vector.tensor_scalar_mul(zh1, in0=z1p, scalar1=rz)
                tpz = ps.tile([128, 384], f16, tag="tp", bufs=1)
                nc.tensor.transpose(tpz[:, 0:128], zh1, ident)
                hpf = small.tile([128, 128], f16, tag="hpf")
                nc.scalar.activation(hpf, tpz[:, 0:128], AF.Relu,
                                     bias=v["p1_be"], scale=v["p1_g"])

                # ================= p2 + final LN =================
                z2p = ps.tile([128, 128], f32, tag="mm", bufs=3)
                nc.tensor.matmul(z2p, hpf, w["p2rhs"], start=True, stop=False)
                nc.tensor.matmul(z2p, ones1, w["p2brow"], start=False,
                                 stop=True)
                v2f = small.tile([128, 1], f32, tag="v2f")
                sqf = small.tile([128, 128], f16, tag="sq")
                nc.scalar.activation(sqf, z2p, AF.Square, accum_out=v2f)
                s2f = small.tile([128, 1], f32, tag="s2f")
                nc.scalar.activation(s2f, v2f, AF.Sqrt, bias=epst,
                                     scale=1.0 / HID)
                r2f = small.tile([128, 1], f32, tag="r2f")
                nc.vector.reciprocal(r2f, s2f)
                tg = small.tile([128, 128], f32, tag="tg")
                nc.vector.scalar_tensor_tensor(
                    out=tg, in0=z2p, scalar=r2f, in1=p2g_bc,
                    op0=OP.mult, op1=OP.mult)
                obuf = small.tile([128, 128], f32, tag="obuf")
                nc.vector.tensor_tensor(out=obuf, in0=tg, in1=p2be_bc,
                                        op=OP.add)
                nc.sync.dma_start(out_d[t * 128:(t + 1) * 128, :], obuf)
    return nc


# ------------------------------------------------------------------ runner
_cache = {}


def _get_nc():
    if "nc" not in _cache:
        import concourse.bacc as bacc
        nc = bacc.Bacc(None, target_bir_lowering=False, debug=False)
        _build(nc)
        nc.compile()
        _cache["nc"] = nc
    return _cache["nc"]


def _install_ntff_shim():
    """Provide antenv.axon_hooks if the image lacks it (profiling only)."""
    import types
    try:
        from antenv.axon_hooks import get_axon_ntff_profile_hook  # noqa
        return
    except ImportError:
        pass
    try:
        sys.path.insert(0, "/root/.axon_site")
        from trn_agent_boot.trn_boot import _ntff_profile_via_ctypes
        hook = _ntff_profile_via_ctypes("/opt/axon/libaxon_pjrt.so")
    except Exception:
        hook = None
    import antenv
    mod = types.ModuleType("antenv.axon_hooks")
    mod.get_axon_ntff_profile_hook = lambda: hook
    mod.set_axon_ntff_profile_hook = lambda h: None
    sys.modules["antenv.axon_hooks"] = mod
    antenv.axon_hooks = mod


def _run(inputs, trace=False):
    if trace:
        _install_ntff_shim()
    from concourse.bass_utils import run_bass_kernel_spmd

    p = {n: np.asarray(inputs[n], np.float32) for n in PARAM_NAMES}
    f = _fold(p)
    cur = np.asarray(inputs["current_node"], np.float32)
    nb = np.asarray(inputs["neighbor_nodes"], np.float32)
    mask = np.asarray(inputs["neighbor_mask"], np.int32)

    in_maps = []
    for i in range(N_CORES):
        d = _prep_shard(cur[i * R:(i + 1) * R], nb[i * R:(i + 1) * R],
                        mask[i * R:(i + 1) * R])
        d.update(f)
        in_maps.append(d)

    nc = _get_nc()
    res = run_bass_kernel_spmd(nc, in_maps, list(range(N_CORES)),
                               trace=trace)
    out = np.concatenate([r["out"] for r in res.results], axis=0)
    return np.ascontiguousarray(out.astype(np.float32)), res


def kernel(**inputs) -> np.ndarray:
    out, _ = _run(inputs, trace=False)
    return out


def kernel_profiled(**inputs):
    """Returns (output, exec_time_ns or None). Used by test.py."""
    out, res = _run(inputs, trace=True)
    return out, res.exec_time_ns
